# revision 27
# baseline (speedup 1.0000x reference)
"""ClsbdCRF message passing on 8 NeuronCores — weight-shift formulation.

Core i handles batch i//2, image-row half i%2 (64 output rows + halo).
Layout: W=128 on partitions, (slots, C, H) on free dims, fp16 compute with
fp32 PSUM accumulation.

msg[p] = sum_t w_t[p] * xp[p + d_t] is re-associated as
u_t[p'] = w_t[p' - d_t] * xp[p']: weights are shifted in COMPACT
[W, 5, 64] per-dy space by PE matmuls whose stationary shift matrices also
fold the +-5/10 compat scales; the dx part of the tap offset is applied in
the product-mul read APs; the dy part is applied by the stationary of the
PE matmuls that accumulate all tap products into one fp32 PSUM accumulator
per output (multi-slice stride-0-out matmuls, start/stop flags).  All DVE
tensor ops keep 16-bit operands at even element offsets so the 2x_1p perf
mode engages (odd offsets measured ~6x slower); odd-aligned taps read from
one-cell-shifted twin tensors produced by Act copies (alignment-agnostic).
"""

import math

import numpy as np

B, C, H, W, D = 4, 21, 128, 128, 5
EPS = 1e-5
HP = 64
HE = HP + 4      # x/s row extent (halo 2)
FE = HP + 8      # feats row extent (halo 4)
BIGPAD = 100.0   # fp16-safe: 5*(100+6)^2 < 65504; exp(-0.5*s) underflows to 0
CP, CN = 10.0, 5.0  # COMPAT_PAIR, COMPAT_CLSBD

DYS = [-2, -1, 0, 1, 2]
POS = {-2: 0, 0: 1, 2: 2, -1: 3, 1: 4}   # in-group slot order: evens, odds
GI = {dy: i for i, dy in enumerate(DYS)}


def slot(dx, dy):
    return 5 * GI[dy] + POS[dx]


DIRTAPS = [(dx, dy) for dx in range(-2, 3) for dy in range(-2, 3)
           if (dx, dy) > (0, 0)]
MIRTAPS = [(dx, dy) for dx in range(-2, 3) for dy in range(-2, 3)
           if (dx, dy) < (0, 0)]

# g1full slot order: directs grouped so mirror matmul reads are affine runs
G1ORD = ([(2, dy) for dy in DYS] + [(1, dy) for dy in DYS]
         + [(0, 1), (0, 2)])
G1SLOT = {t: i for i, t in enumerate(G1ORD)}

RING1 = [(-1, -1), (-1, 0), (-1, 1), (0, -1), (0, 1), (1, -1), (1, 0), (1, 1)]
RING2 = [(-2, -2), (-2, -1), (-2, 0), (-2, 1), (-2, 2), (-1, -2), (-1, 2),
         (0, -2), (0, 2), (1, -2), (1, 2), (2, -2), (2, -1), (2, 0), (2, 1),
         (2, 2)]
EXP1 = [0, 0, 1, 2, 2, 0, 2, 3, 4, 5, 7, 5, 5, 6, 7, 7]
EXP2 = [0, 1, 1, 1, 2, 3, 4, 3, 4, 3, 4, 5, 6, 6, 6, 7]
# r1 stack order: ring1 taps grouped by dy for affine builds
R1ORD = [(-1, -1), (0, -1), (1, -1), (-1, 0), (1, 0), (-1, 1), (0, 1), (1, 1)]
R1MAP = [R1ORD.index(t) for t in RING1]
# fm stack order: ring2 taps sorted by g2-stack slot
FMORD = sorted(range(16), key=lambda k: slot(*RING2[k]))
FMJ = {k: j for j, k in enumerate(FMORD)}

# (dy, out) -> (mul_engine, pair_engine|None); 'v' DVE, 'g' GpSimd;
# pair None = V0 (PE accumulates 5 raw slices)
GROUPS = {(dy, o): ['v', 'v'] for dy in DYS for o in (0, 1)}
for _dy in (-2, -1, 1):
    for _o in (0, 1):
        GROUPS[(_dy, _o)] = ['v', None]
for _o in (0, 1):
    GROUPS[(2, _o)] = ['v', 'v2']

PORD = [0, -1, -2, 1, 2]
BANKS = [(0, 512), (512, 1024), (1024, C * HP)]

_cache = {}


def _runs(pairs):
    """[(dst, src), ...] (dst-sorted) -> [(dst0, src0, ddst, dsrc, n)]."""
    out = []
    i = 0
    while i < len(pairs):
        if i + 1 < len(pairs):
            dd = pairs[i + 1][0] - pairs[i][0]
            ds = pairs[i + 1][1] - pairs[i][1]
            j = i + 1
            while (j + 1 < len(pairs)
                   and pairs[j + 1][0] - pairs[j][0] == dd
                   and pairs[j + 1][1] - pairs[j][1] == ds):
                j += 1
            out.append((pairs[i][0], pairs[i][1], dd, ds, j - i + 1))
            i = j + 1
        else:
            out.append((pairs[i][0], pairs[i][1], 1, 1, 1))
            i += 1
    return out


def _build():
    import concourse.ap as cap
    import concourse.bacc as bacc
    import concourse.mybir as mybir
    from concourse.tile import TileContext

    f16 = mybir.dt.float16
    f32 = mybir.dt.float32
    Act = mybir.ActivationFunctionType
    Alu = mybir.AluOpType

    import os
    dbg = bool(os.environ.get("KDBG"))
    nc = bacc.Bacc()
    x_d = nc.declare_dram_parameter("x", [W, C, HE], f16, isOutput=False)
    sm_d = nc.declare_dram_parameter("smat", [W, 15, W], f16,
                                     isOutput=False)
    sm32_d = nc.declare_dram_parameter("smat32", [W, 4, W], f32,
                                       isOutput=False)
    f_d = nc.declare_dram_parameter("f", [W, D, FE], f16, isOutput=False)
    s_d = nc.declare_dram_parameter("s", [W, HE], f32, isOutput=False)
    o_d = nc.declare_dram_parameter("out", [2, W, C, HP], f32, isOutput=True)
    if dbg:
        dbg_d = {nm: nc.declare_dram_parameter(nm, shp, f16, isOutput=True)
                 for nm, shp in ()}
        dbg_d["d_g2s"] = nc.declare_dram_parameter(
            "d_g2s", [W, 25, HP], f32, isOutput=True)
        dbg_d |= {nm: nc.declare_dram_parameter(nm, shp, f16, isOutput=True)
                 for nm, shp in (("d_g1s", [W, 25, HP]),
                                 
                                 ("d_xp", [W, C, HE]),
                                 ("d_w", [2, 5, W, 5, HP]),
                                 ("d_u", [10, W, 5, C, HP]))}

    def apv(ap, off, dims):
        return cap.AP(ap.tensor, ap.offset + off, [list(ap.ap[0])] + dims)

    # ---- const APs for activation biases (tiny, barrier-covered) ----
    for cname, cval in (("c_eps", EPS), ("c_1eps", 1.0 + EPS), ("c_z", 0.0)):
        ct = nc.alloc_sbuf_tensor(cname, [W, 1], f32)
        nc.gpsimd.memset(ct.ap(), cval)
        nc.const_aps.aps[(f32, cval)] = ct.ap()
    nc.all_engine_barrier()

    with TileContext(nc) as tc:
        with (
            tc.tile_pool(name="io", bufs=1) as io,
            tc.tile_pool(name="up", bufs=4) as up,
            tc.tile_pool(name="tp", bufs=3) as tp,
            tc.tile_pool(name="scr", bufs=2) as scr,
            tc.tile_pool(name="pacc", bufs=1, space="PSUM") as pacc,
            tc.tile_pool(name="pw", bufs=2, space="PSUM") as pw,
        ):
            # ---- loads: s first (rings are the earliest DVE work) ----
            s_t = {0: io.tile([W, HE], f32, name="s0", tag="s0")}
            nc.sync.dma_start(out=s_t[0][:], in_=s_d[:])
            smat = io.tile([W, 15, W], f16, tag="smat")
            nc.scalar.dma_start(out=smat[:], in_=sm_d[:])
            smat32 = io.tile([W, 4, W], f32, tag="smat32")
            nc.sync.dma_start(out=smat32[:], in_=sm32_d[:])
            s132 = {d: smat32[:, i] for i, d in enumerate((-2, -1, 1, 2))}
            s1 = {d: smat[:, GI[d]] for d in DYS}
            m5 = {d: smat[:, 5 + GI[d]] for d in DYS}
            x10 = {d: smat[:, 10 + GI[d]] for d in DYS}
            x16 = io.tile([W, C, HE], f16, tag="x16")
            nc.sync.dma_start(out=x16[:], in_=x_d[:])
            f_all = io.tile([W, 5, D, FE], f16, tag="f_all")
            nc.scalar.dma_start(out=f_all[:, GI[0]], in_=f_d[:])

            # dy-shifts on the PE; zero-filled edges are safe (those
            # weights multiply out-of-image zero xp taps only)
            for dy in (-2, -1, 1, 2):
                ps_ = pw.tile([W, 5, HP], f32, tag="pw")
                psf_ = ps_[:].rearrange("p k h -> p (k h)")
                nc.tensor.matmul(psf_[:, 0:HE], s132[dy], s_t[0][:],
                                 start=True, stop=True)
                st_ = io.tile([W, HE], f32, name=f"ss{dy}", tag=f"ss{dy}")
                nc.vector.tensor_copy(out=st_[:], in_=psf_[:, 0:HE])
                s_t[dy] = st_
            NF = D * FE
            fa_f = f_all[:].rearrange("p k d h -> p (k d h)")
            f_ao = io.tile([W, 5, D, FE], f16, tag="f_ao")
            fo_f = f_ao[:].rearrange("p k d h -> p (k d h)")
            nc.scalar.activation(apv(fo_f, GI[0] * NF, [[1, NF - 1]]),
                                 apv(fa_f, GI[0] * NF + 1, [[1, NF - 1]]),
                                 Act.Copy)
            for dy in (-2, -1, 1, 2):
                pf = pw.tile([W, 5, HP], f32, tag="pw")
                pff = pf[:].rearrange("p k h -> p (k h)")
                nc.tensor.matmul(pff[:, 0:320], s1[dy],
                                 apv(fa_f, GI[0] * NF, [[1, 320]]),
                                 start=True, stop=True)
                pf2 = pw.tile([W, 5, HP], f32, tag="pw")
                pff2 = pf2[:].rearrange("p k h -> p (k h)")
                nc.tensor.matmul(pff2[:, 0:NF - 320], s1[dy],
                                 apv(fa_f, GI[0] * NF + 320, [[1, NF - 320]]),
                                 start=True, stop=True)
                nc.scalar.activation(apv(fa_f, GI[dy] * NF, [[1, 320]]),
                                     pff[:, 0:320], Act.Copy)
                nc.scalar.activation(apv(fa_f, GI[dy] * NF + 320,
                                         [[1, NF - 320]]),
                                     pff2[:, 0:NF - 320], Act.Copy)
                nc.scalar.activation(apv(fo_f, GI[dy] * NF, [[1, NF - 1]]),
                                     apv(fa_f, GI[dy] * NF + 1,
                                         [[1, NF - 1]]),
                                     Act.Copy)

            # ---- clsbd ring max -> g2 stack (fp32, DVE; earliest work) ----
            def s_ap(dx, dy, n=1, stride=1):
                return apv(s_t[dy][:], 2 + dx, [[stride, n], [1, HP]])

            r1 = io.tile([W, 8, HP], f32, tag="r1")
            for gdy in (-1, 0, 1):
                taps = [t for t in R1ORD if t[1] == gdy]
                odds = [t for t in taps if t[0] % 2]
                evens = [t for t in taps if t[0] % 2 == 0]
                if odds:
                    js = [R1ORD.index(t) for t in odds]
                    st = js[1] - js[0] if len(js) > 1 else 1
                    nc.vector.tensor_copy(
                        out=apv(r1[:].rearrange("p k h -> p (k h)"),
                                js[0] * HP, [[st * HP, len(js)], [1, HP]]),
                        in_=s_ap(odds[0][0], gdy, len(odds),
                                 odds[1][0] - odds[0][0] if len(odds) > 1
                                 else 1))
                for t_ in evens:
                    nc.vector.tensor_copy(out=r1[:, R1ORD.index(t_)],
                                          in_=s_ap(t_[0], gdy))

            fm = io.tile([W, 16, HP], f32, tag="fm")
            for j, k in enumerate(FMORD):
                nc.vector.tensor_tensor(
                    out=fm[:, j], in0=r1[:, R1MAP[EXP1[k]]],
                    in1=r1[:, R1MAP[EXP2[k]]], op=Alu.max)

            g2s = io.tile([W, 25, HP], f32, tag="g2s")
            g2s_f = g2s[:].rearrange("p k h -> p (k h)")
            nc.gpsimd.memset(g2s[:, slot(0, 0)], 0.0)
            for dy in (-1, 0, 1):
                taps = [t for t in RING1 if t[1] == dy]
                odds = sorted([t for t in taps if t[0] % 2],
                              key=lambda t: slot(*t))
                evens = [t for t in taps if t[0] % 2 == 0]
                if odds:
                    sl = [slot(*t) for t in odds]
                    dxs = [t[0] for t in odds]
                    nc.vector.tensor_copy(
                        out=apv(g2s_f, sl[0] * HP,
                                [[(sl[1] - sl[0]) * HP if len(sl) > 1
                                  else HP, len(sl)], [1, HP]]),
                        in_=s_ap(dxs[0], dy, len(dxs),
                                 dxs[1] - dxs[0] if len(dxs) > 1 else 1))
                for t_ in evens:
                    nc.vector.tensor_copy(out=g2s[:, slot(*t_)],
                                          in_=s_ap(t_[0], dy))
            for dy in DYS:
                taps = [(dx, d2_) for (dx, d2_) in RING2 if d2_ == dy]
                for par in (0, 1):
                    grp = sorted([t for t in taps if abs(t[0]) % 2 == par],
                                 key=lambda t: slot(*t))
                    if not grp:
                        continue
                    sl = [slot(*t) for t in grp]
                    dxs = [t[0] for t in grp]
                    js = [FMJ[RING2.index(t)] for t in grp]
                    n = len(grp)
                    slst = sl[1] - sl[0] if n > 1 else 1
                    dxst = dxs[1] - dxs[0] if n > 1 else 1
                    jst = js[1] - js[0] if n > 1 else 1
                    nc.vector.tensor_tensor(
                        out=apv(g2s_f, sl[0] * HP, [[slst * HP, n], [1, HP]]),
                        in0=apv(fm[:].rearrange("p k h -> p (k h)"),
                                js[0] * HP, [[jst * HP, n], [1, HP]]),
                        in1=s_ap(dxs[0], dy, n, dxst), op=Alu.max)

            # ---- Ln cluster (one act-table residency) ----
            lnx = io.tile([W, C, HE], f16, tag="lnx")
            nc.scalar.activation(lnx[:], x16[:], Act.Ln, bias=EPS)
            lnn = io.tile([W, 25, HP], f16, tag="lnn")
            nc.scalar.activation(lnn[:], g2s[:], Act.Ln, bias=EPS)
            lnp = io.tile([W, 25, HP], f16, tag="lnp")
            nc.scalar.activation(lnp[:], g2s[:], Act.Ln, bias=1.0 + EPS,
                                 scale=-1.0)

            # ---- polarness -> xp (DVE fp16) ----
            xl = io.tile([W, C, HE], f16, tag="xl")
            nc.vector.tensor_tensor(out=xl[:], in0=x16[:], in1=lnx[:],
                                    op=Alu.mult)
            e10 = scr.tile([W, 10, HE], f16, tag="e10")
            nc.vector.tensor_tensor(out=e10[:], in0=xl[:, 0:10],
                                    in1=xl[:, 10:20], op=Alu.add)
            e5 = scr.tile([W, 5, HE], f16, tag="e5")
            nc.vector.tensor_tensor(out=e5[:], in0=e10[:, 0:5],
                                    in1=e10[:, 5:10], op=Alu.add)
            e2 = scr.tile([W, 2, HE], f16, tag="e2")
            nc.vector.tensor_tensor(out=e2[:], in0=e5[:, 0:2],
                                    in1=e5[:, 2:4], op=Alu.add)
            e1 = scr.tile([W, 2, HE], f16, tag="e1")
            nc.vector.tensor_tensor(out=e1[:, 0], in0=e2[:, 0], in1=e2[:, 1],
                                    op=Alu.add)
            nc.vector.tensor_tensor(out=e1[:, 1], in0=e5[:, 4], in1=xl[:, 20],
                                    op=Alu.add)
            ent = scr.tile([W, HE], f16, tag="ent")
            nc.vector.tensor_tensor(out=ent[:], in0=e1[:, 0], in1=e1[:, 1],
                                    op=Alu.add)
            pl = io.tile([W, HE], f16, tag="pl")
            nc.vector.tensor_scalar(out=pl[:], in0=ent[:],
                                    scalar1=1.0 / math.log(C), scalar2=1.0,
                                    op0=Alu.mult, op1=Alu.add)
            xp = io.tile([W, C, HE], f16, tag="xp")
            nc.vector.tensor_tensor(
                out=xp[:], in0=x16[:],
                in1=pl[:, None, :].broadcast_to((W, C, HE)), op=Alu.mult)
            # ---- pairwise gaussian, batched by dx-run ----
            g1full = io.tile([W, 12, HE], f16, tag="g1full")
            NF = D * FE
            runs_g1 = [  # (g1full slot0, n dys, dy0, src tile, elem offset)
                (10, 2, GI[1], f_all, 2),  # dx=0 taps (0,1),(0,2)
                (0, 5, 0, f_all, 4),       # dx=2: even, offset 2+2
                (5, 5, 0, f_ao, 2),        # dx=1: odd twin
            ]
            for (k0, n, kk0, srct, off) in runs_g1:
                st = srct[:].rearrange("p k d h -> p (k d h)")
                diff = scr.tile([W, n, D, HE], f16, name=f"df{k0}",
                                tag=f"df{n}")
                nc.vector.tensor_tensor(
                    out=diff[:],
                    in0=apv(fa_f, GI[0] * NF + 2,
                            [[0, n], [FE, D], [1, HE]]),
                    in1=apv(st, kk0 * NF + off,
                            [[NF, n], [FE, D], [1, HE]]),
                    op=Alu.subtract)
                sq = scr.tile([W, n, D, HE], f16, name=f"sq{k0}",
                              tag=f"sq{n}")
                nc.vector.tensor_tensor(out=sq[:], in0=diff[:], in1=diff[:],
                                        op=Alu.mult)
                d2 = scr.tile([W, n, 2, HE], f16, name=f"d2{k0}",
                              tag=f"d2{n}")
                nc.vector.tensor_tensor(out=d2[:], in0=sq[:, :, 0:2],
                                        in1=sq[:, :, 2:4], op=Alu.add)
                d1 = scr.tile([W, n, HE], f16, name=f"d1{k0}", tag=f"d1{n}")
                nc.vector.tensor_tensor(out=d1[:], in0=d2[:, :, 0],
                                        in1=d2[:, :, 1], op=Alu.add)
                ssum = scr.tile([W, n, HE], f16, name=f"sm{k0}", tag=f"sm{n}")
                nc.vector.tensor_tensor(out=ssum[:], in0=d1[:],
                                        in1=sq[:, :, 4], op=Alu.add)
                nc.scalar.activation(g1full[:, k0:k0 + n], ssum[:], Act.Exp,
                                     scale=-0.5)

            # ---- g1 stack [W, 25, 64] (plain g1; x10 folded in PE stat) ----
            g1s = io.tile([W, 25, HP], f16, tag="g1s")
            nc.gpsimd.memset(g1s[:, slot(0, 0)], 1.0)
            g1f_f = g1full[:].rearrange("p k h -> p (k h)")
            g1s_f = g1s[:].rearrange("p k h -> p (k h)")
            dir_pairs = sorted((slot(dx, dy), G1SLOT[(dx, dy)] * HE + 2)
                               for (dx, dy) in DIRTAPS)
            for (d0, sr0, dd, ds, n) in _runs(dir_pairs):
                nc.scalar.activation(
                    apv(g1s_f, d0 * HP, [[dd * HP, n], [1, HP]]),
                    apv(g1f_f, sr0, [[ds, n], [1, HP]]), Act.Copy)
            for dym in (-2, -1, 1, 2):
                mirs = sorted(
                    (slot(dx, dym), G1SLOT[(-dx, -dym)] * HE + 2 + dx)
                    for (dx, d2_) in MIRTAPS if d2_ == dym)
                mp = pw.tile([W, 5, HP], f32, tag="pw")
                mpf = mp[:].rearrange("p k h -> p (k h)")
                col = 0
                segs = []
                for (d0, sr0, dd, ds, n) in _runs(mirs):
                    nc.tensor.matmul(
                        mpf[:, col:col + n * HP], s1[dym],
                        apv(g1f_f, sr0, [[ds, n], [1, HP]]),
                        start=True, stop=True)
                    segs.append((d0, dd, n, col))
                    col += n * HP
                for (d0, dd, n, c0) in segs:
                    nc.scalar.activation(
                        apv(g1s_f, d0 * HP, [[dd * HP, n], [1, HP]]),
                        mpf[:, c0:c0 + n * HP], Act.Copy)
            for dx in (-1, -2):
                nc.scalar.activation(
                    g1s[:, slot(dx, 0)],
                    g1full[:, G1SLOT[(-dx, 0)], 2 + dx:2 + dx + HP], Act.Copy)

            # ---- weights: PE shift+scale per dy ----
            wsh = {}
            for dy in PORD:
                g0 = 5 * GI[dy] * HP
                pn = pw.tile([W, 5, HP], f32, tag="pw")
                pnf = pn[:].rearrange("p k h -> p (k h)")
                # wsh[m] = wn[m - dy]  ->  stationary shift by -dy
                nc.tensor.matmul(pnf[:], m5[-dy],
                                 apv(lnn[:].rearrange("p k h -> p (k h)"),
                                     g0, [[1, 5 * HP]]),
                                 start=True, stop=False)
                nc.tensor.matmul(pnf[:], x10[-dy],
                                 apv(g1s_f, g0, [[1, 5 * HP]]),
                                 start=False, stop=True)
                wn = io.tile([W, 5, HP], f16, name=f"wn{dy}", tag=f"wn{dy}")
                nc.scalar.activation(wn[:], pn[:], Act.Copy)
                pp = pw.tile([W, 5, HP], f32, tag="pw")
                ppf = pp[:].rearrange("p k h -> p (k h)")
                nc.tensor.matmul(ppf[:], m5[-dy],
                                 apv(lnp[:].rearrange("p k h -> p (k h)"),
                                     g0, [[1, 5 * HP]]),
                                 start=True, stop=True)
                wp = io.tile([W, 5, HP], f16, name=f"wp{dy}", tag=f"wp{dy}")
                nc.scalar.activation(wp[:], pp[:], Act.Copy)
                wsh[(dy, 0)], wsh[(dy, 1)] = wn, wp
                if dy == PORD[0]:
                    xpo = io.tile([W, C, HE], f16, tag="xpo")
                    nc.scalar.activation(xpo[:, :, 0:HE - 1], xp[:, :, 1:HE],
                                         Act.Copy)
                if dbg:
                    nc.sync.dma_start(out=dbg_d["d_w"][0, GI[dy]], in_=wn[:])
                    nc.sync.dma_start(out=dbg_d["d_w"][1, GI[dy]], in_=wp[:])

            # ---- products (DVE muls; pairs DVE or V0->PE) + accumulation ----
            acc = [pacc.tile([W, C, HP], f32, name=f"acc{o}", tag=f"acc{o}")
                   for o in (0, 1)]
            accf = [a[:].rearrange("p c h -> p (c h)") for a in acc]
            for gi_, dy in enumerate(PORD):
                for o in (0, 1):
                    _, pe_ = GROUPS[(dy, o)]
                    w = wsh[(dy, o)]
                    wf = w[:].rearrange("p k h -> p (k h)")
                    u = up.tile([W, 5, C, HP], f16, tag="u")
                    nc.vector.tensor_tensor(
                        out=u[:, 0:3],
                        in0=apv(xp[:], 0, [[2, 3], [HE, C], [1, HP]]),
                        in1=apv(wf, 0, [[HP, 3], [0, C], [1, HP]]),
                        op=Alu.mult)
                    nc.vector.tensor_tensor(
                        out=u[:, 3:5],
                        in0=apv(xpo[:], 0, [[2, 2], [HE, C], [1, HP]]),
                        in1=apv(wf, 3 * HP, [[HP, 2], [0, C], [1, HP]]),
                        op=Alu.mult)
                    if dbg:
                        nc.sync.dma_start(out=dbg_d["d_u"][2 * gi_ + o],
                                          in_=u[:])
                    uf = u[:].rearrange("p s c h -> p s (c h)")
                    first, last = gi_ == 0, gi_ == len(PORD) - 1
                    if pe_ == 'v2':
                        t = tp.tile([W, 2, C, HP], f16, tag="t")
                        nc.vector.tensor_tensor(
                            out=t[:], in0=u[:, 0:4:2], in1=u[:, 1:4:2],
                            op=Alu.add)
                        t2 = tp.tile([W, C, HP], f16, tag="t2")
                        nc.vector.tensor_tensor(
                            out=t2[:], in0=t[:, 0], in1=t[:, 1], op=Alu.add)
                        av = tp.tile([W, C, HP], f16, tag="av")
                        nc.vector.tensor_tensor(
                            out=av[:], in0=t2[:], in1=u[:, 4], op=Alu.add)
                        avf = av[:].rearrange("p c h -> p (c h)")[:, None, :]
                        slices = [(avf, 1)]
                    elif pe_ is not None:
                        t = tp.tile([W, 2, C, HP], f16, tag="t")
                        nc.vector.tensor_tensor(
                            out=t[:], in0=u[:, 0:4:2], in1=u[:, 1:4:2],
                            op=Alu.add)
                        tf = t[:].rearrange("p s c h -> p s (c h)")
                        slices = [(tf, 2), (uf, 1)]
                    else:
                        slices = [(uf[:, 0:4], 4), (uf, 1)]
                    for (n0, n1) in BANKS:
                        for si, (sv, ns) in enumerate(slices):
                            sub = max(1, 512 // ns) if ns > 1 else 512
                            for m0 in range(n0, n1, sub):
                                m1 = min(m0 + sub, n1)
                                if ns == 1:
                                    mv = sv[:, sv.shape[1] - 1, m0:m1]
                                    ov = accf[o][:, m0:m1]
                                else:
                                    mv = sv[:, 0:ns, m0:m1]
                                    ov = accf[o][:, None, m0:m1] \
                                        .broadcast_to((W, ns, m1 - m0))
                                st = first and si == 0 and m0 == n0
                                sp = last and si == len(slices) - 1 \
                                    and m1 == n1
                                nc.tensor.matmul(ov, s1[dy], mv,
                                                 start=st, stop=sp)
            if dbg:
                nc.sync.dma_start(out=dbg_d["d_g1s"][:], in_=g1s[:])
                nc.sync.dma_start(out=dbg_d["d_g2s"][:], in_=g2s[:])
                nc.sync.dma_start(out=dbg_d["d_xp"][:], in_=xp[:])
            res0 = io.tile([W, C, HP], f32, tag="res0")
            res1 = io.tile([W, C, HP], f32, tag="res1")
            r0f = res0[:].rearrange("p c h -> p (c h)")
            r1f = res1[:].rearrange("p c h -> p (c h)")
            od_f = [o_d[o].rearrange("p c h -> p (c h)") for o in (0, 1)]
            for (n0, n1) in BANKS:
                nc.scalar.activation(r0f[:, n0:n1], accf[0][:, n0:n1],
                                     Act.Copy)
                nc.sync.dma_start(out=od_f[0][:, n0:n1], in_=r0f[:, n0:n1])
                nc.scalar.activation(r1f[:, n0:n1], accf[1][:, n0:n1],
                                     Act.Copy)
                nc.sync.dma_start(out=od_f[1][:, n0:n1], in_=r1f[:, n0:n1])
    nc.finalize()
    return nc


_last_results = None


def kernel(input, feats, clsbd_feats, label=None, **_ignored):
    global _last_results
    from concourse.bass_utils import run_bass_kernel_spmd

    x = np.asarray(input, np.float32)
    f = np.asarray(feats, np.float32)
    s = np.asarray(clsbd_feats, np.float32)

    xpad = np.zeros((B, C, H + 4, W), np.float16)
    xpad[:, :, 2:2 + H] = x.astype(np.float16)
    smat32 = np.zeros((W, 4, W), np.float32)
    for i, d in enumerate((-2, -1, 1, 2)):
        for k in range(W):
            if 0 <= k - d < W:
                smat32[k, i, k - d] = 1.0
    smat = np.zeros((W, 15, W), np.float16)
    for d in DYS:
        for k in range(W):
            m = k - d
            if 0 <= m < W:
                smat[k, GI[d], m] = 1.0
                smat[k, 5 + GI[d], m] = -CN
                smat[k, 10 + GI[d], m] = CP
    fpad = np.full((B, D, H + 8, W), BIGPAD, np.float16)
    fpad[:, :, 4:4 + H] = f.astype(np.float16)
    spad = np.zeros((B, H + 4, W), np.float32)
    spad[:, 2:2 + H] = s[:, 0]

    in_maps = []
    for i in range(8):
        b, half = i // 2, i % 2
        h0 = half * HP
        in_maps.append({
            "x": np.ascontiguousarray(
                xpad[b, :, h0:h0 + HE].transpose(2, 0, 1)),
            "smat": smat,
            "smat32": smat32,
            "f": np.ascontiguousarray(
                fpad[b, :, h0:h0 + FE].transpose(2, 0, 1)),
            "s": np.ascontiguousarray(spad[b, h0:h0 + HE].transpose(1, 0)),
        })

    if "nc" not in _cache:
        _cache["nc"] = _build()
    res = run_bass_kernel_spmd(_cache["nc"], in_maps, list(range(8)))
    _last_results = res

    out = np.empty((2, B, C, H, W), np.float32)
    for i in range(8):
        b, half = i // 2, i % 2
        h0 = half * HP
        out[:, b, :, h0:h0 + HP] = res.results[i]["out"].transpose(0, 2, 3, 1)
    return out


# revision 28
# speedup vs baseline: 1.0021x; 1.0021x over previous
"""ClsbdCRF message passing on 8 NeuronCores — weight-shift formulation.

Core i handles batch i//2, image-row half i%2 (64 output rows + halo).
Layout: W=128 on partitions, (slots, C, H) on free dims, fp16 compute with
fp32 PSUM accumulation.

msg[p] = sum_t w_t[p] * xp[p + d_t] is re-associated as
u_t[p'] = w_t[p' - d_t] * xp[p']: weights are shifted in COMPACT
[W, 5, 64] per-dy space by PE matmuls whose stationary shift matrices also
fold the +-5/10 compat scales; the dx part of the tap offset is applied in
the product-mul read APs; the dy part is applied by the stationary of the
PE matmuls that accumulate all tap products into one fp32 PSUM accumulator
per output (multi-slice stride-0-out matmuls, start/stop flags).  All DVE
tensor ops keep 16-bit operands at even element offsets so the 2x_1p perf
mode engages (odd offsets measured ~6x slower); odd-aligned taps read from
one-cell-shifted twin tensors produced by Act copies (alignment-agnostic).
"""

import math

import numpy as np

B, C, H, W, D = 4, 21, 128, 128, 5
EPS = 1e-5
HP = 64
HE = HP + 4      # x/s row extent (halo 2)
FE = HP + 8      # feats row extent (halo 4)
BIGPAD = 100.0   # fp16-safe: 5*(100+6)^2 < 65504; exp(-0.5*s) underflows to 0
CP, CN = 10.0, 5.0  # COMPAT_PAIR, COMPAT_CLSBD

DYS = [-2, -1, 0, 1, 2]
POS = {-2: 0, 0: 1, 2: 2, -1: 3, 1: 4}   # in-group slot order: evens, odds
GI = {dy: i for i, dy in enumerate(DYS)}


def slot(dx, dy):
    return 5 * GI[dy] + POS[dx]


DIRTAPS = [(dx, dy) for dx in range(-2, 3) for dy in range(-2, 3)
           if (dx, dy) > (0, 0)]
MIRTAPS = [(dx, dy) for dx in range(-2, 3) for dy in range(-2, 3)
           if (dx, dy) < (0, 0)]

# g1full slot order: directs grouped so mirror matmul reads are affine runs
G1ORD = ([(2, dy) for dy in DYS] + [(1, dy) for dy in DYS]
         + [(0, 1), (0, 2)])
G1SLOT = {t: i for i, t in enumerate(G1ORD)}

RING1 = [(-1, -1), (-1, 0), (-1, 1), (0, -1), (0, 1), (1, -1), (1, 0), (1, 1)]
RING2 = [(-2, -2), (-2, -1), (-2, 0), (-2, 1), (-2, 2), (-1, -2), (-1, 2),
         (0, -2), (0, 2), (1, -2), (1, 2), (2, -2), (2, -1), (2, 0), (2, 1),
         (2, 2)]
EXP1 = [0, 0, 1, 2, 2, 0, 2, 3, 4, 5, 7, 5, 5, 6, 7, 7]
EXP2 = [0, 1, 1, 1, 2, 3, 4, 3, 4, 3, 4, 5, 6, 6, 6, 7]
# r1 stack order: ring1 taps grouped by dy for affine builds
R1ORD = [(-1, -1), (0, -1), (1, -1), (-1, 0), (1, 0), (-1, 1), (0, 1), (1, 1)]
R1MAP = [R1ORD.index(t) for t in RING1]
# fm stack order: ring2 taps sorted by g2-stack slot
FMORD = sorted(range(16), key=lambda k: slot(*RING2[k]))
FMJ = {k: j for j, k in enumerate(FMORD)}

# (dy, out) -> (mul_engine, pair_engine|None); 'v' DVE, 'g' GpSimd;
# pair None = V0 (PE accumulates 5 raw slices)
GROUPS = {(dy, o): ['v', 'v'] for dy in DYS for o in (0, 1)}
for _dy in (-2, -1, 1):
    for _o in (0, 1):
        GROUPS[(_dy, _o)] = ['v', None]
for _o in (0, 1):
    GROUPS[(2, _o)] = ['v', 'v2']

PORD = [0, 1, -1, -2, 2]
BANKS = [(0, 512), (512, 1024), (1024, C * HP)]

_cache = {}


def _runs(pairs):
    """[(dst, src), ...] (dst-sorted) -> [(dst0, src0, ddst, dsrc, n)]."""
    out = []
    i = 0
    while i < len(pairs):
        if i + 1 < len(pairs):
            dd = pairs[i + 1][0] - pairs[i][0]
            ds = pairs[i + 1][1] - pairs[i][1]
            j = i + 1
            while (j + 1 < len(pairs)
                   and pairs[j + 1][0] - pairs[j][0] == dd
                   and pairs[j + 1][1] - pairs[j][1] == ds):
                j += 1
            out.append((pairs[i][0], pairs[i][1], dd, ds, j - i + 1))
            i = j + 1
        else:
            out.append((pairs[i][0], pairs[i][1], 1, 1, 1))
            i += 1
    return out


def _build():
    import concourse.ap as cap
    import concourse.bacc as bacc
    import concourse.mybir as mybir
    from concourse.tile import TileContext

    f16 = mybir.dt.float16
    f32 = mybir.dt.float32
    Act = mybir.ActivationFunctionType
    Alu = mybir.AluOpType

    import os
    dbg = bool(os.environ.get("KDBG"))
    nc = bacc.Bacc()
    x_d = nc.declare_dram_parameter("x", [W, C, HE], f16, isOutput=False)
    sm_d = nc.declare_dram_parameter("smat", [W, 15, W], f16,
                                     isOutput=False)
    sm32_d = nc.declare_dram_parameter("smat32", [W, 4, W], f32,
                                       isOutput=False)
    f_d = nc.declare_dram_parameter("f", [W, D, FE], f16, isOutput=False)
    s_d = nc.declare_dram_parameter("s", [W, HE], f32, isOutput=False)
    o_d = nc.declare_dram_parameter("out", [2, W, C, HP], f32, isOutput=True)
    if dbg:
        dbg_d = {nm: nc.declare_dram_parameter(nm, shp, f16, isOutput=True)
                 for nm, shp in ()}
        dbg_d["d_g2s"] = nc.declare_dram_parameter(
            "d_g2s", [W, 25, HP], f32, isOutput=True)
        dbg_d |= {nm: nc.declare_dram_parameter(nm, shp, f16, isOutput=True)
                 for nm, shp in (("d_g1s", [W, 25, HP]),
                                 
                                 ("d_xp", [W, C, HE]),
                                 ("d_w", [2, 5, W, 5, HP]),
                                 ("d_u", [10, W, 5, C, HP]))}

    def apv(ap, off, dims):
        return cap.AP(ap.tensor, ap.offset + off, [list(ap.ap[0])] + dims)

    # ---- const APs for activation biases (tiny, barrier-covered) ----
    for cname, cval in (("c_eps", EPS), ("c_1eps", 1.0 + EPS), ("c_z", 0.0)):
        ct = nc.alloc_sbuf_tensor(cname, [W, 1], f32)
        nc.gpsimd.memset(ct.ap(), cval)
        nc.const_aps.aps[(f32, cval)] = ct.ap()
    nc.all_engine_barrier()

    with TileContext(nc) as tc:
        with (
            tc.tile_pool(name="io", bufs=1) as io,
            tc.tile_pool(name="up", bufs=4) as up,
            tc.tile_pool(name="tp", bufs=3) as tp,
            tc.tile_pool(name="scr", bufs=2) as scr,
            tc.tile_pool(name="pacc", bufs=1, space="PSUM") as pacc,
            tc.tile_pool(name="pw", bufs=2, space="PSUM") as pw,
        ):
            # ---- loads: s first (rings are the earliest DVE work) ----
            s_t = {0: io.tile([W, HE], f32, name="s0", tag="s0")}
            nc.sync.dma_start(out=s_t[0][:], in_=s_d[:])
            smat = io.tile([W, 15, W], f16, tag="smat")
            nc.scalar.dma_start(out=smat[:], in_=sm_d[:])
            smat32 = io.tile([W, 4, W], f32, tag="smat32")
            nc.sync.dma_start(out=smat32[:], in_=sm32_d[:])
            s132 = {d: smat32[:, i] for i, d in enumerate((-2, -1, 1, 2))}
            s1 = {d: smat[:, GI[d]] for d in DYS}
            m5 = {d: smat[:, 5 + GI[d]] for d in DYS}
            x10 = {d: smat[:, 10 + GI[d]] for d in DYS}
            x16 = io.tile([W, C, HE], f16, tag="x16")
            nc.sync.dma_start(out=x16[:], in_=x_d[:])
            f_all = io.tile([W, 5, D, FE], f16, tag="f_all")
            nc.scalar.dma_start(out=f_all[:, GI[0]], in_=f_d[:])

            # dy-shifts on the PE; zero-filled edges are safe (those
            # weights multiply out-of-image zero xp taps only)
            for dy in (-2, -1, 1, 2):
                ps_ = pw.tile([W, 5, HP], f32, tag="pw")
                psf_ = ps_[:].rearrange("p k h -> p (k h)")
                nc.tensor.matmul(psf_[:, 0:HE], s132[dy], s_t[0][:],
                                 start=True, stop=True)
                st_ = io.tile([W, HE], f32, name=f"ss{dy}", tag=f"ss{dy}")
                nc.vector.tensor_copy(out=st_[:], in_=psf_[:, 0:HE])
                s_t[dy] = st_
            NF = D * FE
            fa_f = f_all[:].rearrange("p k d h -> p (k d h)")
            f_ao = io.tile([W, 5, D, FE], f16, tag="f_ao")
            fo_f = f_ao[:].rearrange("p k d h -> p (k d h)")
            nc.scalar.activation(apv(fo_f, GI[0] * NF, [[1, NF - 1]]),
                                 apv(fa_f, GI[0] * NF + 1, [[1, NF - 1]]),
                                 Act.Copy)
            for dy in (-2, -1, 1, 2):
                pf = pw.tile([W, 5, HP], f32, tag="pw")
                pff = pf[:].rearrange("p k h -> p (k h)")
                nc.tensor.matmul(pff[:, 0:320], s1[dy],
                                 apv(fa_f, GI[0] * NF, [[1, 320]]),
                                 start=True, stop=True)
                pf2 = pw.tile([W, 5, HP], f32, tag="pw")
                pff2 = pf2[:].rearrange("p k h -> p (k h)")
                nc.tensor.matmul(pff2[:, 0:NF - 320], s1[dy],
                                 apv(fa_f, GI[0] * NF + 320, [[1, NF - 320]]),
                                 start=True, stop=True)
                nc.scalar.activation(apv(fa_f, GI[dy] * NF, [[1, 320]]),
                                     pff[:, 0:320], Act.Copy)
                nc.scalar.activation(apv(fa_f, GI[dy] * NF + 320,
                                         [[1, NF - 320]]),
                                     pff2[:, 0:NF - 320], Act.Copy)
                nc.scalar.activation(apv(fo_f, GI[dy] * NF, [[1, NF - 1]]),
                                     apv(fa_f, GI[dy] * NF + 1,
                                         [[1, NF - 1]]),
                                     Act.Copy)

            # ---- clsbd ring max -> g2 stack (fp32, DVE; earliest work) ----
            def s_ap(dx, dy, n=1, stride=1):
                return apv(s_t[dy][:], 2 + dx, [[stride, n], [1, HP]])

            r1 = io.tile([W, 8, HP], f32, tag="r1")
            for gdy in (-1, 0, 1):
                taps = [t for t in R1ORD if t[1] == gdy]
                odds = [t for t in taps if t[0] % 2]
                evens = [t for t in taps if t[0] % 2 == 0]
                if odds:
                    js = [R1ORD.index(t) for t in odds]
                    st = js[1] - js[0] if len(js) > 1 else 1
                    nc.vector.tensor_copy(
                        out=apv(r1[:].rearrange("p k h -> p (k h)"),
                                js[0] * HP, [[st * HP, len(js)], [1, HP]]),
                        in_=s_ap(odds[0][0], gdy, len(odds),
                                 odds[1][0] - odds[0][0] if len(odds) > 1
                                 else 1))
                for t_ in evens:
                    nc.vector.tensor_copy(out=r1[:, R1ORD.index(t_)],
                                          in_=s_ap(t_[0], gdy))

            fm = io.tile([W, 16, HP], f32, tag="fm")
            for j, k in enumerate(FMORD):
                nc.vector.tensor_tensor(
                    out=fm[:, j], in0=r1[:, R1MAP[EXP1[k]]],
                    in1=r1[:, R1MAP[EXP2[k]]], op=Alu.max)

            g2s = io.tile([W, 25, HP], f32, tag="g2s")
            g2s_f = g2s[:].rearrange("p k h -> p (k h)")
            nc.gpsimd.memset(g2s[:, slot(0, 0)], 0.0)
            for dy in (-1, 0, 1):
                taps = [t for t in RING1 if t[1] == dy]
                odds = sorted([t for t in taps if t[0] % 2],
                              key=lambda t: slot(*t))
                evens = [t for t in taps if t[0] % 2 == 0]
                if odds:
                    sl = [slot(*t) for t in odds]
                    dxs = [t[0] for t in odds]
                    nc.vector.tensor_copy(
                        out=apv(g2s_f, sl[0] * HP,
                                [[(sl[1] - sl[0]) * HP if len(sl) > 1
                                  else HP, len(sl)], [1, HP]]),
                        in_=s_ap(dxs[0], dy, len(dxs),
                                 dxs[1] - dxs[0] if len(dxs) > 1 else 1))
                for t_ in evens:
                    nc.vector.tensor_copy(out=g2s[:, slot(*t_)],
                                          in_=s_ap(t_[0], dy))
            for dy in DYS:
                taps = [(dx, d2_) for (dx, d2_) in RING2 if d2_ == dy]
                for par in (0, 1):
                    grp = sorted([t for t in taps if abs(t[0]) % 2 == par],
                                 key=lambda t: slot(*t))
                    if not grp:
                        continue
                    sl = [slot(*t) for t in grp]
                    dxs = [t[0] for t in grp]
                    js = [FMJ[RING2.index(t)] for t in grp]
                    n = len(grp)
                    slst = sl[1] - sl[0] if n > 1 else 1
                    dxst = dxs[1] - dxs[0] if n > 1 else 1
                    jst = js[1] - js[0] if n > 1 else 1
                    nc.vector.tensor_tensor(
                        out=apv(g2s_f, sl[0] * HP, [[slst * HP, n], [1, HP]]),
                        in0=apv(fm[:].rearrange("p k h -> p (k h)"),
                                js[0] * HP, [[jst * HP, n], [1, HP]]),
                        in1=s_ap(dxs[0], dy, n, dxst), op=Alu.max)

            # ---- Ln cluster (one act-table residency) ----
            lnx = io.tile([W, C, HE], f16, tag="lnx")
            nc.scalar.activation(lnx[:], x16[:], Act.Ln, bias=EPS)
            lnn = io.tile([W, 25, HP], f16, tag="lnn")
            nc.scalar.activation(lnn[:], g2s[:], Act.Ln, bias=EPS)
            lnp = io.tile([W, 25, HP], f16, tag="lnp")
            nc.scalar.activation(lnp[:], g2s[:], Act.Ln, bias=1.0 + EPS,
                                 scale=-1.0)

            # ---- polarness -> xp (DVE fp16) ----
            xl = io.tile([W, C, HE], f16, tag="xl")
            nc.vector.tensor_tensor(out=xl[:], in0=x16[:], in1=lnx[:],
                                    op=Alu.mult)
            e10 = scr.tile([W, 10, HE], f16, tag="e10")
            nc.vector.tensor_tensor(out=e10[:], in0=xl[:, 0:10],
                                    in1=xl[:, 10:20], op=Alu.add)
            e5 = scr.tile([W, 5, HE], f16, tag="e5")
            nc.vector.tensor_tensor(out=e5[:], in0=e10[:, 0:5],
                                    in1=e10[:, 5:10], op=Alu.add)
            e2 = scr.tile([W, 2, HE], f16, tag="e2")
            nc.vector.tensor_tensor(out=e2[:], in0=e5[:, 0:2],
                                    in1=e5[:, 2:4], op=Alu.add)
            e1 = scr.tile([W, 2, HE], f16, tag="e1")
            nc.vector.tensor_tensor(out=e1[:, 0], in0=e2[:, 0], in1=e2[:, 1],
                                    op=Alu.add)
            nc.vector.tensor_tensor(out=e1[:, 1], in0=e5[:, 4], in1=xl[:, 20],
                                    op=Alu.add)
            ent = scr.tile([W, HE], f16, tag="ent")
            nc.vector.tensor_tensor(out=ent[:], in0=e1[:, 0], in1=e1[:, 1],
                                    op=Alu.add)
            pl = io.tile([W, HE], f16, tag="pl")
            nc.vector.tensor_scalar(out=pl[:], in0=ent[:],
                                    scalar1=1.0 / math.log(C), scalar2=1.0,
                                    op0=Alu.mult, op1=Alu.add)
            xp = io.tile([W, C, HE], f16, tag="xp")
            nc.vector.tensor_tensor(
                out=xp[:], in0=x16[:],
                in1=pl[:, None, :].broadcast_to((W, C, HE)), op=Alu.mult)
            # ---- pairwise gaussian, batched by dx-run ----
            g1full = io.tile([W, 12, HE], f16, tag="g1full")
            NF = D * FE
            runs_g1 = [  # (g1full slot0, n dys, dy0, src tile, elem offset)
                (10, 2, GI[1], f_all, 2),  # dx=0 taps (0,1),(0,2)
                (0, 5, 0, f_all, 4),       # dx=2: even, offset 2+2
                (5, 5, 0, f_ao, 2),        # dx=1: odd twin
            ]
            for (k0, n, kk0, srct, off) in runs_g1:
                st = srct[:].rearrange("p k d h -> p (k d h)")
                diff = scr.tile([W, n, D, HE], f16, name=f"df{k0}",
                                tag=f"df{n}")
                nc.vector.tensor_tensor(
                    out=diff[:],
                    in0=apv(fa_f, GI[0] * NF + 2,
                            [[0, n], [FE, D], [1, HE]]),
                    in1=apv(st, kk0 * NF + off,
                            [[NF, n], [FE, D], [1, HE]]),
                    op=Alu.subtract)
                sq = scr.tile([W, n, D, HE], f16, name=f"sq{k0}",
                              tag=f"sq{n}")
                nc.vector.tensor_tensor(out=sq[:], in0=diff[:], in1=diff[:],
                                        op=Alu.mult)
                d2 = scr.tile([W, n, 2, HE], f16, name=f"d2{k0}",
                              tag=f"d2{n}")
                nc.vector.tensor_tensor(out=d2[:], in0=sq[:, :, 0:2],
                                        in1=sq[:, :, 2:4], op=Alu.add)
                d1 = scr.tile([W, n, HE], f16, name=f"d1{k0}", tag=f"d1{n}")
                nc.vector.tensor_tensor(out=d1[:], in0=d2[:, :, 0],
                                        in1=d2[:, :, 1], op=Alu.add)
                ssum = scr.tile([W, n, HE], f16, name=f"sm{k0}", tag=f"sm{n}")
                nc.vector.tensor_tensor(out=ssum[:], in0=d1[:],
                                        in1=sq[:, :, 4], op=Alu.add)
                nc.scalar.activation(g1full[:, k0:k0 + n], ssum[:], Act.Exp,
                                     scale=-0.5)

            # ---- g1 stack [W, 25, 64] (plain g1; x10 folded in PE stat) ----
            g1s = io.tile([W, 25, HP], f16, tag="g1s")
            nc.gpsimd.memset(g1s[:, slot(0, 0)], 1.0)
            g1f_f = g1full[:].rearrange("p k h -> p (k h)")
            g1s_f = g1s[:].rearrange("p k h -> p (k h)")
            dir_pairs = sorted((slot(dx, dy), G1SLOT[(dx, dy)] * HE + 2)
                               for (dx, dy) in DIRTAPS)
            for (d0, sr0, dd, ds, n) in _runs(dir_pairs):
                nc.scalar.activation(
                    apv(g1s_f, d0 * HP, [[dd * HP, n], [1, HP]]),
                    apv(g1f_f, sr0, [[ds, n], [1, HP]]), Act.Copy)
            for dym in (-2, -1, 1, 2):
                mirs = sorted(
                    (slot(dx, dym), G1SLOT[(-dx, -dym)] * HE + 2 + dx)
                    for (dx, d2_) in MIRTAPS if d2_ == dym)
                mp = pw.tile([W, 5, HP], f32, tag="pw")
                mpf = mp[:].rearrange("p k h -> p (k h)")
                col = 0
                segs = []
                for (d0, sr0, dd, ds, n) in _runs(mirs):
                    nc.tensor.matmul(
                        mpf[:, col:col + n * HP], s1[dym],
                        apv(g1f_f, sr0, [[ds, n], [1, HP]]),
                        start=True, stop=True)
                    segs.append((d0, dd, n, col))
                    col += n * HP
                for (d0, dd, n, c0) in segs:
                    nc.scalar.activation(
                        apv(g1s_f, d0 * HP, [[dd * HP, n], [1, HP]]),
                        mpf[:, c0:c0 + n * HP], Act.Copy)
            for dx in (-1, -2):
                nc.scalar.activation(
                    g1s[:, slot(dx, 0)],
                    g1full[:, G1SLOT[(-dx, 0)], 2 + dx:2 + dx + HP], Act.Copy)

            # ---- weights: PE shift+scale per dy ----
            wsh = {}
            for dy in PORD:
                g0 = 5 * GI[dy] * HP
                pn = pw.tile([W, 5, HP], f32, tag="pw")
                pnf = pn[:].rearrange("p k h -> p (k h)")
                # wsh[m] = wn[m - dy]  ->  stationary shift by -dy
                nc.tensor.matmul(pnf[:], m5[-dy],
                                 apv(lnn[:].rearrange("p k h -> p (k h)"),
                                     g0, [[1, 5 * HP]]),
                                 start=True, stop=False)
                nc.tensor.matmul(pnf[:], x10[-dy],
                                 apv(g1s_f, g0, [[1, 5 * HP]]),
                                 start=False, stop=True)
                wn = io.tile([W, 5, HP], f16, name=f"wn{dy}", tag=f"wn{dy}")
                nc.scalar.activation(wn[:], pn[:], Act.Copy)
                pp = pw.tile([W, 5, HP], f32, tag="pw")
                ppf = pp[:].rearrange("p k h -> p (k h)")
                nc.tensor.matmul(ppf[:], m5[-dy],
                                 apv(lnp[:].rearrange("p k h -> p (k h)"),
                                     g0, [[1, 5 * HP]]),
                                 start=True, stop=True)
                wp = io.tile([W, 5, HP], f16, name=f"wp{dy}", tag=f"wp{dy}")
                nc.scalar.activation(wp[:], pp[:], Act.Copy)
                wsh[(dy, 0)], wsh[(dy, 1)] = wn, wp
                if dy == PORD[0]:
                    xpo = io.tile([W, C, HE], f16, tag="xpo")
                    nc.scalar.activation(xpo[:, :, 0:HE - 1], xp[:, :, 1:HE],
                                         Act.Copy)
                if dbg:
                    nc.sync.dma_start(out=dbg_d["d_w"][0, GI[dy]], in_=wn[:])
                    nc.sync.dma_start(out=dbg_d["d_w"][1, GI[dy]], in_=wp[:])

            # ---- products (DVE muls; pairs DVE or V0->PE) + accumulation ----
            acc = [pacc.tile([W, C, HP], f32, name=f"acc{o}", tag=f"acc{o}")
                   for o in (0, 1)]
            accf = [a[:].rearrange("p c h -> p (c h)") for a in acc]
            for gi_, dy in enumerate(PORD):
                for o in (0, 1):
                    _, pe_ = GROUPS[(dy, o)]
                    w = wsh[(dy, o)]
                    wf = w[:].rearrange("p k h -> p (k h)")
                    u = up.tile([W, 5, C, HP], f16, tag="u")
                    nc.vector.tensor_tensor(
                        out=u[:, 0:3],
                        in0=apv(xp[:], 0, [[2, 3], [HE, C], [1, HP]]),
                        in1=apv(wf, 0, [[HP, 3], [0, C], [1, HP]]),
                        op=Alu.mult)
                    nc.vector.tensor_tensor(
                        out=u[:, 3:5],
                        in0=apv(xpo[:], 0, [[2, 2], [HE, C], [1, HP]]),
                        in1=apv(wf, 3 * HP, [[HP, 2], [0, C], [1, HP]]),
                        op=Alu.mult)
                    if dbg:
                        nc.sync.dma_start(out=dbg_d["d_u"][2 * gi_ + o],
                                          in_=u[:])
                    uf = u[:].rearrange("p s c h -> p s (c h)")
                    first, last = gi_ == 0, gi_ == len(PORD) - 1
                    if pe_ == 'v2':
                        t = tp.tile([W, 2, C, HP], f16, tag="t")
                        nc.vector.tensor_tensor(
                            out=t[:], in0=u[:, 0:4:2], in1=u[:, 1:4:2],
                            op=Alu.add)
                        t2 = tp.tile([W, C, HP], f16, tag="t2")
                        nc.vector.tensor_tensor(
                            out=t2[:], in0=t[:, 0], in1=t[:, 1], op=Alu.add)
                        av = tp.tile([W, C, HP], f16, tag="av")
                        nc.vector.tensor_tensor(
                            out=av[:], in0=t2[:], in1=u[:, 4], op=Alu.add)
                        avf = av[:].rearrange("p c h -> p (c h)")[:, None, :]
                        slices = [(avf, 1)]
                    elif pe_ is not None:
                        t = tp.tile([W, 2, C, HP], f16, tag="t")
                        nc.vector.tensor_tensor(
                            out=t[:], in0=u[:, 0:4:2], in1=u[:, 1:4:2],
                            op=Alu.add)
                        tf = t[:].rearrange("p s c h -> p s (c h)")
                        slices = [(tf, 2), (uf, 1)]
                    else:
                        slices = [(uf[:, 0:4], 4), (uf, 1)]
                    for (n0, n1) in BANKS:
                        for si, (sv, ns) in enumerate(slices):
                            sub = max(1, 512 // ns) if ns > 1 else 512
                            for m0 in range(n0, n1, sub):
                                m1 = min(m0 + sub, n1)
                                if ns == 1:
                                    mv = sv[:, sv.shape[1] - 1, m0:m1]
                                    ov = accf[o][:, m0:m1]
                                else:
                                    mv = sv[:, 0:ns, m0:m1]
                                    ov = accf[o][:, None, m0:m1] \
                                        .broadcast_to((W, ns, m1 - m0))
                                st = first and si == 0 and m0 == n0
                                sp = last and si == len(slices) - 1 \
                                    and m1 == n1
                                nc.tensor.matmul(ov, s1[dy], mv,
                                                 start=st, stop=sp)
            if dbg:
                nc.sync.dma_start(out=dbg_d["d_g1s"][:], in_=g1s[:])
                nc.sync.dma_start(out=dbg_d["d_g2s"][:], in_=g2s[:])
                nc.sync.dma_start(out=dbg_d["d_xp"][:], in_=xp[:])
            res0 = io.tile([W, C, HP], f32, tag="res0")
            res1 = io.tile([W, C, HP], f32, tag="res1")
            r0f = res0[:].rearrange("p c h -> p (c h)")
            r1f = res1[:].rearrange("p c h -> p (c h)")
            od_f = [o_d[o].rearrange("p c h -> p (c h)") for o in (0, 1)]
            for (n0, n1) in BANKS:
                nc.scalar.activation(r0f[:, n0:n1], accf[0][:, n0:n1],
                                     Act.Copy)
                nc.sync.dma_start(out=od_f[0][:, n0:n1], in_=r0f[:, n0:n1])
                nc.scalar.activation(r1f[:, n0:n1], accf[1][:, n0:n1],
                                     Act.Copy)
                nc.sync.dma_start(out=od_f[1][:, n0:n1], in_=r1f[:, n0:n1])
    nc.finalize()
    return nc


_last_results = None


def kernel(input, feats, clsbd_feats, label=None, **_ignored):
    global _last_results
    from concourse.bass_utils import run_bass_kernel_spmd

    x = np.asarray(input, np.float32)
    f = np.asarray(feats, np.float32)
    s = np.asarray(clsbd_feats, np.float32)

    xpad = np.zeros((B, C, H + 4, W), np.float16)
    xpad[:, :, 2:2 + H] = x.astype(np.float16)
    smat32 = np.zeros((W, 4, W), np.float32)
    for i, d in enumerate((-2, -1, 1, 2)):
        for k in range(W):
            if 0 <= k - d < W:
                smat32[k, i, k - d] = 1.0
    smat = np.zeros((W, 15, W), np.float16)
    for d in DYS:
        for k in range(W):
            m = k - d
            if 0 <= m < W:
                smat[k, GI[d], m] = 1.0
                smat[k, 5 + GI[d], m] = -CN
                smat[k, 10 + GI[d], m] = CP
    fpad = np.full((B, D, H + 8, W), BIGPAD, np.float16)
    fpad[:, :, 4:4 + H] = f.astype(np.float16)
    spad = np.zeros((B, H + 4, W), np.float32)
    spad[:, 2:2 + H] = s[:, 0]

    in_maps = []
    for i in range(8):
        b, half = i // 2, i % 2
        h0 = half * HP
        in_maps.append({
            "x": np.ascontiguousarray(
                xpad[b, :, h0:h0 + HE].transpose(2, 0, 1)),
            "smat": smat,
            "smat32": smat32,
            "f": np.ascontiguousarray(
                fpad[b, :, h0:h0 + FE].transpose(2, 0, 1)),
            "s": np.ascontiguousarray(spad[b, h0:h0 + HE].transpose(1, 0)),
        })

    if "nc" not in _cache:
        _cache["nc"] = _build()
    res = run_bass_kernel_spmd(_cache["nc"], in_maps, list(range(8)))
    _last_results = res

    out = np.empty((2, B, C, H, W), np.float32)
    for i in range(8):
        b, half = i // 2, i % 2
        h0 = half * HP
        out[:, b, :, h0:h0 + HP] = res.results[i]["out"].transpose(0, 2, 3, 1)
    return out
